# revision 13
# baseline (speedup 1.0000x reference)
"""Trainium2 Bass kernel for the BiDirectionalRNN problem.

Math (matches the fp32 jax reference):
    e = emb[x]                                   # [B, T, 512]
    fwd:  h_t = relu(e_t @ Wf.T + bf + h_{t-1})  # fs[t]
    bwd over reversed e: bs[s]                   # generation order
    xcat = concat_t [fs[t], bs[t]]  -> [B, T*1024]
    h1 = relu(xcat @ W1.T + b1); 4x h = relu(h @ W2.T + b2); out = h @ Wo.T + bo

Strategy:
  * Data-parallel over batch: 1024/8 = 128 samples per NeuronCore.
  * Host folds embedding + input projection weights into per-direction
    tables WfeB = Wf @ emb.T + bf ([512, 97]). The device builds the
    one-hot of x on the fly (a rank-1 matmul replicates the x row over 97
    partitions, DVE is_equal against an arange column), then computes the
    per-step drive terms a = WfeB @ onehot with K=97 matmuls. Only the
    FWD one-hot is built; the bwd direction reuses it and reverses t in
    the PSUM->SBUF copy (negative-stride destination AP).
  * ScalarE copies each a-GEMM PSUM block into the scan layout
    [p, b*33 + s] (strided 3D AP), separator column = -1e30.
  * The whole 32-step recurrence h = relu(a + h_prev) runs as ONE DVE
    tensor_tensor_scan per (dir, hid-tile): state=(a add state) max 0,
    fp32 internal state; the separator resets state to 0 between chains.
  * W1 (64MB fp32) ships as fp8-e4m3 (16MB/core, halving the dominant
    HBM stream vs bf16). Quantization error is tamed two ways:
      - a global power-of-2 scale (absmax -> ~224) applied on the host
        and undone for free by the h1 ReLU's per-partition scale operand
        (one scale column rides in IDA), and
      - error-feedback rounding along t (k-rows for adjacent timesteps
        multiply nearly identical RNN states h_t ~ h_t+1, so pushing each
        row's rounding error into the next-lower t cancels most of the
        output error). Measured end-to-end rel-err ~1.2e-2 (vs 8.5e-3 for
        the all-bf16 kernel, gate 2e-2).
  * The scan-output states are split h = hi + lo into TWO fp8-e4m3 tiles
    (lo = fp8(h - fp8(h))), recovering full bf16 activation precision
    while keeping both GEMM operands fp8. The split copies also
    transpose to t-major [p, t*128+b] (strided DVE APs) because the
    DoubleRow weight load requires [128, 2, 128] with an aligned pair
    stride and contiguous columns (s3_lw_dual_fp8_restrictions).
  * The [B,32768]@[32768,512] GEMM runs as DoubleRow fp8 matmuls: each
    instruction contracts a PAIR of 128-row k-chunks (lhsT [128,2,128] =
    two adjacent t-slices of hi/lo, rhs [128,2,512] = the matching
    pair-interleaved W1 chunk). hi and lo passes reuse the same W1 tile
    (no extra DMA) and all accumulate into one PSUM bank.
  * W1 ships in 32 [128,4096] fp8 groups (4 t-pairs each), (dir, m)-major
    so group G only needs scan j = G//4; a-phases are software-pipelined
    two steps ahead. The first two groups are issued before the consts;
    the final group is fetched pair-by-pair to minimize end latency.
  * Tail: PE-transpose h1, then 4 x [512,512] + [97,512] in transposed
    (feature-major) layout; biases enter PSUM via rank-1 matmuls; each
    stage uses twin PSUM banks so ScalarE and VectorE drain in parallel.
"""

import numpy as np
import ml_dtypes

BF16 = ml_dtypes.bfloat16
F8E4 = ml_dtypes.float8_e4m3

MOD = 97
HID = 512
T = 32
B = 1024
NCORES = 8
BL = B // NCORES          # 128 batch per core
CL = T + 1                # chain length incl. separator column
FREE = BL * CL            # 4224 scan columns per tile
NEG = -1e30
W1_GRP = 32               # W1 DMA groups of 4 k-chunk-pairs (512KB each)
FB_GAMMA = 0.95           # error-feedback strength for W1 quantization

_CACHE: dict = {}


def _build(reps=1):
    import concourse.tile as tile
    from concourse import bacc, mybir

    fp32 = mybir.dt.float32
    bf16 = mybir.dt.bfloat16
    f8e4 = mybir.dt.float8e4

    nc = bacc.Bacc(
        "TRN2", target_bir_lowering=False, debug=False, num_devices=NCORES
    )

    d = {
        "IDA": nc.dram_tensor("IDA", [128, 130], bf16, kind="ExternalInput").ap(),
        "WFE": nc.dram_tensor("WFE", [MOD, 2 * HID], bf16, kind="ExternalInput").ap(),
        "W1S": nc.dram_tensor("W1S", [W1_GRP, 128, 4096], f8e4, kind="ExternalInput").ap(),
        "W2O": nc.dram_tensor("W2O", [128, 4 * 512 + 4 * MOD], bf16, kind="ExternalInput").ap(),
        "BIA": nc.dram_tensor("BIA", [1, 1121 + BL * T], bf16, kind="ExternalInput").ap(),
        "OUT": nc.dram_tensor("OUT", [MOD, BL], fp32, kind="ExternalOutput").ap(),
    }

    with tile.TileContext(nc) as tc:
        for _ in range(reps):
            _emit(tc, d, mybir)

    nc.compile()
    return nc


def _emit(tc, d, mybir):
    nc = tc.nc
    fp32 = mybir.dt.float32
    bf16 = mybir.dt.bfloat16
    f8e4 = mybir.dt.float8e4
    AF = mybir.ActivationFunctionType
    ALU = mybir.AluOpType
    DR = mybir.MatmulPerfMode.DoubleRow

    from contextlib import ExitStack

    with ExitStack() as ctx:
        const = ctx.enter_context(tc.tile_pool(name="const", bufs=1))
        a_pool = ctx.enter_context(tc.tile_pool(name="apool", bufs=2))
        h_pool = ctx.enter_context(tc.tile_pool(name="hpool", bufs=3))
        s_pool = ctx.enter_context(tc.tile_pool(name="spool", bufs=6))
        w1_pool = ctx.enter_context(tc.tile_pool(name="w1pool", bufs=16))
        hp_pool = ctx.enter_context(tc.tile_pool(name="hppool", bufs=3))
        ps_a = ctx.enter_context(tc.tile_pool(name="psa", bufs=2, space="PSUM"))
        ps_h1 = ctx.enter_context(tc.tile_pool(name="psh1", bufs=1, space="PSUM"))
        ps_t = ctx.enter_context(tc.tile_pool(name="pst", bufs=1, space="PSUM"))
        ps_l = ctx.enter_context(tc.tile_pool(name="psl", bufs=1, space="PSUM"))
        ps_o = ctx.enter_context(tc.tile_pool(name="pso", bufs=1, space="PSUM"))

        # ---- head: start the W1 stream before anything else ----
        # HWDGE descriptor generations serialize (~0.6us each); issuing the
        # first two W1 groups first keeps the DMA engines busy while the
        # const descriptors generate (W1 g0 isn't consumed until ~6us).
        w1_pre = {}
        for G in (0, 1):
            w_t = w1_pool.tile([128, 4096], f8e4, tag="w_t")
            nc.sync.dma_start(w_t[:], d["W1S"][G])
            w1_pre[G] = w_t

        # ---- constants (merged DMAs to avoid early DMA-engine bubbles) ----
        wfe = const.tile([MOD, 2 * HID], bf16)
        nc.sync.dma_start(wfe[:], d["WFE"][:])
        w2o = const.tile([128, 4 * 512 + 4 * MOD], bf16)
        w2sb = w2o[:, 0:2048]
        wosb = w2o[:, 2048:2048 + 4 * MOD]
        bia = const.tile([1, 1121 + BL * T], bf16)
        nc.sync.dma_start(bia[:], d["BIA"])
        b1sb = bia[:, 0:512]
        b2r = bia[:, 512:1024]
        bor = bia[:, 1024:1121]
        xr = bia[:, 1121:1121 + BL * T]
        ida = const.tile([128, 130], bf16)
        nc.sync.dma_start(ida[:], d["IDA"])
        idsb = ida[:, 0:128]
        arn = ida[:, 128:129]
        # 1/s (global W1 scale) as fp32: the Activation scale AP must be FP32
        invs = const.tile([128, 1], fp32)
        nc.vector.tensor_copy(invs[:], ida[:, 129:130])
        ones = const.tile([1, 128], bf16)
        nc.vector.memset(ones[:], 1.0)
        zero = const.tile([128, 1], bf16)
        nc.vector.memset(zero[:], 0.0)
        # one-hot of x (fwd order only), built on device: replicate the x
        # row over 97 partitions with a rank-1 matmul, compare with arange
        ohsb = const.tile([MOD, BL * T], bf16)

        # ---- drive terms + scans + linear1, interleaved per j = dir*4 + m ----
        # a = WfeB @ onehot in 8 PSUM blocks of 16 chains; ScalarE lays each
        # block into the scan layout [p, b*33 + s] (t reversed for the bwd
        # direction); the DVE scan computes h = relu(a + h_prev) for all 128
        # chains in one instruction; DVE then splits h into fp8 hi/lo; the
        # four W1 pair-groups for this j stream in and accumulate (hi + lo
        # DoubleRow passes) into psum_h1.
        psum_h1 = ps_h1.tile([128, 512], fp32)
        bias_done = [False]

        def a_phase(j):
            dd, m = j // 4, j % 4
            a_sb = a_pool.tile([128, FREE], bf16, tag="a")
            sep = a_sb[:].rearrange("p (b t) -> p b t", t=CL)[:, :, T]
            nc.vector.memset(sep, NEG)
            lhsT = wfe[:, dd * HID + m * 128: dd * HID + m * 128 + 128]
            for q in range(8):
                if j == 0:
                    px = ps_a.tile([128, 512], fp32, tag="pa")
                    nc.tensor.matmul(
                        px[:MOD, :], ones[:, 0:MOD],
                        xr[:, q * 512: (q + 1) * 512],
                        start=True, stop=True,
                    )
                    nc.vector.tensor_tensor(
                        ohsb[:, q * 512:(q + 1) * 512], px[:MOD, :],
                        arn[:MOD, :].broadcast_to([MOD, 512]),
                        op=mybir.AluOpType.is_equal,
                    )
                pa = ps_a.tile([128, 512], fp32, tag="pa")
                nc.tensor.matmul(
                    pa[:], lhsT, ohsb[:, q * 512:(q + 1) * 512],
                    start=True, stop=True,
                )
                av = a_sb[:].rearrange("p (b t) -> p b t", t=CL)[:, 16 * q:16 * q + 16, 0:T]
                if dd:
                    av = av[:, :, ::-1]
                nc.scalar.copy(av, pa[:].rearrange("p (b t) -> p b t", t=T))
            h_t = h_pool.tile([128, FREE], bf16, tag="h")
            nc.vector.tensor_tensor_scan(
                h_t[:], a_sb[:], zero[:].broadcast_to([128, FREE]),
                initial=0.0, op0=ALU.add, op1=ALU.max,
            )
            # split h = hi + lo into two fp8 operand tiles for DoubleRow,
            # transposing to t-major [p, t*128 + b] (drops the separator)
            hview = h_t[:].rearrange("p (b t) -> p t b", t=CL)[:, 0:T, :]
            hi_t = s_pool.tile([128, T * BL], f8e4, tag="hi")
            hi3 = hi_t[:].rearrange("p (t b) -> p t b", b=BL)
            nc.vector.tensor_copy(hi3, hview)
            lo_t = s_pool.tile([128, T * BL], f8e4, tag="lo")
            lo3 = lo_t[:].rearrange("p (t b) -> p t b", b=BL)
            nc.vector.tensor_tensor(lo3, hview, hi3, op=ALU.subtract)
            return hi_t, lo_t

        hs = {0: a_phase(0), 1: a_phase(1)}
        for j in range(8):
            hi_t, lo_t = hs[j]
            for G in range(4 * j, 4 * j + 4):
                w_t = w1_pre.pop(G, None)
                if w_t is None:
                    w_t = w1_pool.tile([128, 4096], f8e4, tag="w_t")
                last_grp = G == W1_GRP - 1
                if last_grp:
                    # taper: fetch the final group pair-by-pair so the
                    # last matmuls only wait on their own 128KB slice
                    for c in range(4):
                        nc.sync.dma_start(w_t[:, c * 1024:(c + 1) * 1024],
                                          d["W1S"][G][:, c * 1024:(c + 1) * 1024])
                elif G > 1:
                    nc.sync.dma_start(w_t[:], d["W1S"][G])
                if not bias_done[0]:
                    # rank-1 bias opens the accumulation: ones.T @ b1
                    # broadcasts b1 over the batch partitions
                    nc.tensor.matmul(psum_h1[:], ones[:], b1sb,
                                     start=True, stop=False)
                    bias_done[0] = True
                for c in range(4):
                    pp = (G % 4) * 4 + c          # local t-pair index
                    wv = w_t[:, c * 1024:(c + 1) * 1024].rearrange(
                        "p (two n) -> p two n", two=2)
                    hiv = hi_t[:, 2 * pp * BL:(2 * pp + 2) * BL].rearrange(
                        "p (two b) -> p two b", two=2)
                    lov = lo_t[:, 2 * pp * BL:(2 * pp + 2) * BL].rearrange(
                        "p (two b) -> p two b", two=2)
                    nc.tensor.matmul(
                        psum_h1[:], hiv, wv,
                        start=False, stop=False, perf_mode=DR,
                    )
                    nc.tensor.matmul(
                        psum_h1[:], lov, wv,
                        start=False, stop=(last_grp and c == 3), perf_mode=DR,
                    )
                if G == 4 * j and j + 2 < 8:
                    hs[j + 2] = a_phase(j + 2)
        # tail-only weights ship after the W1 stream so the last W1 byte
        # (the critical one) arrives earlier; this DMA overlaps the h1
        # drain + transposes and lands before the first layer matmul
        nc.sync.dma_start(w2o[:], d["W2O"][:])
        h1sb = const.tile([128, 512], bf16)
        # the per-partition scale undoes the global W1 quantization scale:
        # relu(psum * (1/s)) == relu(psum)/s for s > 0
        nc.scalar.activation(h1sb[:], psum_h1[:], AF.Relu, scale=invs[:])

        # ---- transpose h1 to feature-major [512, 128] ----
        # Twin PSUM banks per stage: ScalarE drains one while VectorE drains
        # the other (Tile serializes same-bank readers, so one bank can't be
        # split across engines).
        pt_a = ps_t.tile([128, 256], bf16, tag="pta")
        pt_b = ps_t.tile([128, 256], bf16, tag="ptb")
        cur = hp_pool.tile([128, 512], bf16, tag="hp")
        for m in (0, 1):
            nc.tensor.transpose(
                pt_a[:, (m % 2) * 128:(m % 2) * 128 + 128],
                h1sb[:, m * 128:(m + 1) * 128], idsb[:])
        nc.scalar.copy(cur[:, 0:256], pt_a[:])
        for m in (2, 3):
            nc.tensor.transpose(
                pt_b[:, (m % 2) * 128:(m % 2) * 128 + 128],
                h1sb[:, m * 128:(m + 1) * 128], idsb[:])
        nc.vector.tensor_copy(cur[:, 256:512], pt_b[:])

        # ---- 4 x (h = relu(W2 @ h' + b2)), feature-major, col block = m ----
        for _L in range(4):
            pl_a = ps_l.tile([128, 256], fp32, tag="pla")
            pl_b = ps_l.tile([128, 256], fp32, tag="plb")
            for m in range(4):
                pl = pl_a if m < 2 else pl_b
                col = (m % 2) * 128
                nc.tensor.matmul(
                    pl[:, col:col + 128],
                    b2r[:, m * 128:(m + 1) * 128], ones[:],
                    start=True, stop=False,
                )
                for k in range(4):
                    nc.tensor.matmul(
                        pl[:, col:col + 128],
                        w2sb[:, k * 512 + m * 128: k * 512 + m * 128 + 128],
                        cur[:, k * 128:(k + 1) * 128],
                        start=False, stop=(k == 3),
                    )
            hq = hp_pool.tile([128, 512], bf16, tag="hp")
            nc.scalar.activation(hq[:, 0:256], pl_a[:], AF.Relu)
            nc.vector.tensor_scalar_max(hq[:, 256:512], pl_b[:], 0.0)
            cur = hq

        # ---- output head: out' = Wo @ h' + bo  -> [97, 128] ----
        po = ps_o.tile([MOD, 128], fp32, tag="po")
        nc.tensor.matmul(po[:], bor, ones[:], start=True, stop=False)
        for k in range(4):
            nc.tensor.matmul(
                po[:], wosb[:, k * MOD:(k + 1) * MOD], cur[:, k * 128:(k + 1) * 128],
                start=False, stop=(k == 3),
            )
        osb = const.tile([MOD, BL], fp32)
        nc.scalar.copy(osb[:], po[:])
        nc.sync.dma_start(d["OUT"], osb[:])


def _quant_w1_feedback(W1T, s):
    """e4m3-quantize the scaled W1.T with error feedback along t.

    k-rows for adjacent timesteps multiply nearly identical activations
    (h_t ~ h_{t+1}), so pushing each row's rounding error into the
    next-lower-t row cancels most of the GEMM output error. Sweeps t
    downward so the residual lands on t=0 (the smallest |h|).
    """
    W = (W1T * s).reshape(T, 2 * HID, HID).astype(np.float32)
    Q = np.empty((T, 2 * HID, HID), dtype=F8E4)
    e = np.zeros((2 * HID, HID), np.float32)
    for t in range(T - 1, -1, -1):
        v = W[t] + FB_GAMMA * e
        qv = v.astype(F8E4)
        e = v - qv.astype(np.float32)
        Q[t] = qv
    return Q.reshape(2 * T * HID, HID)


def _host_prep(inputs):
    x = np.asarray(inputs["x"]).astype(np.int64)          # [B, T]
    emb = np.asarray(inputs["emb"], np.float32)           # [97, 512]
    Wf = np.asarray(inputs["Wf"], np.float32)
    bf = np.asarray(inputs["bf"], np.float32)
    Wb = np.asarray(inputs["Wb"], np.float32)
    bb = np.asarray(inputs["bb"], np.float32)
    W1 = np.asarray(inputs["W1"], np.float32)             # [512, 32768]
    b1 = np.asarray(inputs["b1"], np.float32)
    W2 = np.asarray(inputs["W2"], np.float32)
    b2 = np.asarray(inputs["b2"], np.float32)
    Wo = np.asarray(inputs["Wo"], np.float32)             # [97, 512]
    bo = np.asarray(inputs["bo"], np.float32)

    # fold embedding gather + input projection + bias:
    # a_d[:, b, s] = (Wd @ emb.T + bd)[:, idx] since onehot has exactly one 1
    WFE = np.ascontiguousarray(np.stack([
        (Wf @ emb.T + bf[:, None]).T,                     # [97, 512]
        (Wb @ emb.T + bb[:, None]).T,
    ]).transpose(1, 0, 2).reshape(MOD, 2 * HID)).astype(BF16)

    # per-core x rows (fwd order only; bwd reuses it with t reversed on
    # device), col = b*32 + s; values 0..96 are exact in bf16
    xc = x.reshape(NCORES, BL, T)
    XR = xc.reshape(NCORES, BL * T).astype(BF16)          # [NC, 4096]

    # W1 -> fp8-e4m3 with a global pow2 scale (undone on device by the h1
    # ReLU's per-partition scale = 1/s) and error-feedback rounding along
    # t; layout [32, 128, 4096]: group G = (d, m, gg); within a group, 4
    # t-pairs side by side, each pair interleaved [p, 2, n] for DoubleRow.
    W1T = W1.T.astype(np.float32)                         # [32768, 512] rows t-major
    s = float(2.0 ** np.floor(np.log2(224.0 / np.abs(W1T).max())))
    W1q = _quant_w1_feedback(W1T, s)                      # [32768, 512] e4m3, scaled
    W1S = np.ascontiguousarray(
        W1q.reshape(4, 4, 2, 2, 4, 128, 512)     # [gg, pig, i, d, m, p, col]
        .transpose(3, 4, 0, 5, 1, 2, 6)          # [d, m, gg, p, pig, i, col]
        .reshape(W1_GRP, 128, 4096)
    )
    IDA = np.concatenate([
        np.eye(128, dtype=np.float32),
        np.arange(128, dtype=np.float32).reshape(128, 1),
        np.full((128, 1), 1.0 / s, dtype=np.float32),
    ], axis=1).astype(BF16)
    W2S = np.ascontiguousarray(W2.T.reshape(4, 128, 512).transpose(1, 0, 2).reshape(128, 2048)).astype(BF16)
    WOS = np.ascontiguousarray(Wo.T.reshape(4, 128, MOD).transpose(1, 0, 2).reshape(128, 4 * MOD)).astype(BF16)
    W2O = np.concatenate([W2S, WOS], axis=1)
    BIAH = np.concatenate([b1 * s, b2, bo]).astype(BF16)  # [1121]

    shared = {"WFE": WFE, "W1S": W1S, "W2O": W2O, "IDA": IDA}
    in_maps = [
        dict(shared, BIA=np.concatenate([BIAH, XR[c]]).reshape(1, -1))
        for c in range(NCORES)
    ]
    return in_maps


def _get_nc():
    if "nc" not in _CACHE:
        _CACHE["nc"] = _build()
    return _CACHE["nc"]


def kernel(**inputs):
    from concourse.bass_utils import run_bass_kernel_spmd

    nc = _get_nc()
    in_maps = _host_prep(inputs)
    res = run_bass_kernel_spmd(nc, in_maps, list(range(NCORES)))
    outs = [np.asarray(res.results[c]["OUT"], np.float32) for c in range(NCORES)]
    return np.ascontiguousarray(np.concatenate([o.T for o in outs], axis=0))  # [1024, 97]
